# revision 6
# baseline (speedup 1.0000x reference)
"""TRN2 Bass LSTM kernel — batch-data-parallel across 8 NeuronCores.

Problem: nn_CCellBaseLSTM (T=2048, B=64, I=H=512).
  out[t] = h_t of a standard LSTM; returns (out, (h_T, c_T)) like the reference.

Design (per core, B=8 batch lanes, everything "transposed" so gate rows /
hidden units sit on SBUF partitions):
- Input GEMM is chunked over time (TC=128): xg.T = w_ih' @ x.T streamed as
  bf16 matmuls with the weights stationary, bias folded into the psum->SBUF
  copy (DVE tensor_scalar add with a per-partition bias operand).
- Recurrence: gates.T [2048,8] per step as 64 (LDWEIGHTS+MATMUL) pairs with
  K=128, M=128, N=8, accumulating 4 K-tiles per gate row tile into PSUM.
  PSUM is split across two banks (i,f,g vs o) so the activation chain can
  start while the o-gate matmuls finish.
- Elementwise: ACT does sigmoid/tanh, DVE does the c/h updates in f32;
  h is written as bf16 (matmul operand dtype) and DMA'd out per step.
- Host does all layout transforms (weight transpose/reshape, x transpose to
  [I, T, B], output unpermute) — only device time counts.
"""
import os
import sys

for _p in ("/opt/trn_rl_repo", "/root/.axon_site/_ro/trn_rl_repo"):
    if os.path.isdir(_p) and _p not in sys.path:
        sys.path.insert(0, _p)

import numpy as np
import ml_dtypes

T, BTOT, I, H = 2048, 64, 512, 512
N_CORES = 8
B = BTOT // N_CORES          # 8 batch lanes per core
TC = 128                     # timesteps per chunk
NCHUNK = T // TC
BF = ml_dtypes.bfloat16

_compiled_nc = None


def _build():
    import concourse.mybir as mybir
    import concourse.tile as tile
    from concourse import bacc

    F32 = mybir.dt.float32
    BF16 = mybir.dt.bfloat16
    AF = mybir.ActivationFunctionType
    NCOL = TC * B

    nc = bacc.Bacc("TRN2", target_bir_lowering=False, debug=False,
                   num_devices=N_CORES)
    d_whh = nc.dram_tensor("lhsT_hh", [128, 4, 2048], BF16, kind="ExternalInput")
    d_wih = nc.dram_tensor("lhsT_ih", [128, 4, 2048], BF16, kind="ExternalInput")
    d_bias = nc.dram_tensor("bias", [128, 16], F32, kind="ExternalInput")
    d_xT = nc.dram_tensor("xT", [NCHUNK, 128, 4, NCOL], BF16, kind="ExternalInput")
    d_oh = nc.dram_tensor("out_h", [T, 128, 4, B], BF16, kind="ExternalOutput")
    d_oc = nc.dram_tensor("out_c", [128, 4, B], F32, kind="ExternalOutput")

    HDEPTH = 4  # h ring depth (lets PE run ahead of the act chain)
    with tile.TileContext(nc) as tc:
        with (
            tc.tile_pool(name="const", bufs=1) as const,
            tc.tile_pool(name="state", bufs=1) as state,
            tc.tile_pool(name="xin", bufs=2) as xpool,
            tc.tile_pool(name="xg", bufs=2) as xgpool,
            tc.tile_pool(name="psx", bufs=2, space="PSUM") as psx,
            tc.tile_pool(name="psg", bufs=2, space="PSUM") as psg,
            tc.tile_pool(name="work", bufs=4) as work,
        ):
            whh = const.tile([128, 4, 2048], BF16)
            wih = const.tile([128, 4, 2048], BF16)
            bias = const.tile([128, 16], F32)
            nc.sync.dma_start(whh[:], d_whh[:])
            nc.sync.dma_start(wih[:], d_wih[:])
            nc.sync.dma_start(bias[:], d_bias[:])

            cT = state.tile([128, 2, 4, B], F32)   # ping-pong c state
            hT = state.tile([128, HDEPTH, 4, B], BF16)
            nc.vector.memset(cT[:], 0.0)
            nc.vector.memset(hT[:], 0.0)

            NGRP = 32  # x-GEMM (j, s) groups per chunk (16 M-tiles x 2 halves)
            xstate = {}

            def emit_xgemm_group(xt_t, xg_t, gidx):
                j, s = gidx // 2, (gidx % 2) * 512
                w = min(512, NCOL - s)
                pt = psx.tile([128, 512], F32, name="xq_pt", tag="xq_pt")
                for k in range(4):
                    nc.tensor.matmul(
                        pt[:, :w],
                        lhsT=wih[:, k, 128 * j:128 * (j + 1)],
                        rhs=xt_t[:, k, s:s + w],
                        start=(k == 0), stop=(k == 3),
                    )
                nc.vector.tensor_scalar_add(
                    xg_t[:, j, s:s + w], pt[:, :w], bias[:, j:j + 1])

            def emit_xgemm_quarter(xt_t, xg_t, qidx):
                """One K-slice of one (j, s) group: spreads a group's 4 MMs
                over 4 successive steps to avoid bursty PE injection."""
                gidx, k = qidx // 4, qidx % 4
                if gidx >= NGRP:
                    return
                j, s = gidx // 2, (gidx % 2) * 512
                w = min(512, NCOL - s)
                if k == 0:
                    xstate["pt"] = psx.tile([128, 512], F32, name="xq_pt",
                                            tag="xq_pt")
                pt = xstate["pt"]
                nc.tensor.matmul(
                    pt[:, :w],
                    lhsT=wih[:, k, 128 * j:128 * (j + 1)],
                    rhs=xt_t[:, k, s:s + w],
                    start=(k == 0), stop=(k == 3),
                )
                if k == 3:
                    nc.vector.tensor_scalar_add(
                        xg_t[:, j, s:s + w], pt[:, :w], bias[:, j:j + 1])

            # prologue: chunk 0 input GEMM up front
            cur_xt = xpool.tile([128, 4, NCOL], BF16, tag="xt")
            nc.sync.dma_start(cur_xt[:], d_xT[0])
            cur_xg = xgpool.tile([128, 16, NCOL], BF16, tag="xg")
            for g in range(NGRP):
                emit_xgemm_group(cur_xt, cur_xg, g)

            for n in range(NCHUNK):
                xg = cur_xg
                nxt_xt = nxt_xg = None
                if n + 1 < NCHUNK:
                    nxt_xt = xpool.tile([128, 4, NCOL], BF16, tag="xt")
                    nc.sync.dma_start(nxt_xt[:], d_xT[n + 1])
                    nxt_xg = xgpool.tile([128, 16, NCOL], BF16, tag="xg")

                for tt in range(TC):
                    t = n * TC + tt
                    cur = t % HDEPTH
                    prv = (t - 1) % HDEPTH
                    # gate order in M-tiles: i=0:4, f=4:8, g=8:12, o=12:16
                    def mm_block(dst, jlist):
                        for ji, j in enumerate(jlist):
                            for k in range(4):
                                nc.tensor.matmul(
                                    dst[:, ji, :],
                                    lhsT=whh[:, k, 128 * j:128 * (j + 1)],
                                    rhs=hT[:, prv, k, :],
                                    start=(k == 0), stop=(k == 3),
                                )
                    xgs = xg[:, :, tt * B:(tt + 1) * B]
                    # g-gates first in their own PSUM bank so tanh(g)
                    # overlaps the i/f matmuls; o last (only needed for h)
                    pg_g = psg.tile([128, 4, B], F32, tag="pg_g")
                    mm_block(pg_g, [8, 9, 10, 11])
                    gpre = work.tile([128, 4, B], F32, tag="gpre")
                    nc.vector.tensor_tensor(
                        gpre[:], pg_g[:], xgs[:, 8:12, :], mybir.AluOpType.add)
                    gg = work.tile([128, 4, B], F32, tag="gg")
                    nc.scalar.activation(gg[:], gpre[:], AF.Tanh)
                    pg_if = psg.tile([128, 8, B], F32, tag="pg_if")
                    mm_block(pg_if, [0, 1, 2, 3, 4, 5, 6, 7])
                    pre = work.tile([128, 8, B], F32, tag="pre")
                    nc.vector.tensor_tensor(
                        pre[:], pg_if[:], xgs[:, 0:8, :], mybir.AluOpType.add)
                    sig = work.tile([128, 8, B], F32, tag="sig")
                    nc.scalar.activation(sig[:], pre[:], AF.Sigmoid)
                    pg_o = psg.tile([128, 4, B], F32, tag="pg_o")
                    mm_block(pg_o, [12, 13, 14, 15])
                    # next chunk's input GEMM, spread 1 matmul per step and
                    # emitted after the h-dependent matmuls so it fills PE
                    # idle during the activation chain
                    if nxt_xg is not None:
                        emit_xgemm_quarter(nxt_xt, nxt_xg, tt)
                    preo = work.tile([128, 4, B], F32, tag="preo")
                    nc.vector.tensor_tensor(
                        preo[:], pg_o[:], xgs[:, 12:16, :], mybir.AluOpType.add)
                    sigo = work.tile([128, 4, B], F32, tag="sigo")
                    nc.scalar.activation(sigo[:], preo[:], AF.Sigmoid)
                    cc, cp = t % 2, 1 - (t % 2)   # c ping-pong slots
                    t1 = work.tile([128, 4, B], F32, tag="t1")
                    nc.vector.tensor_mul(t1[:], sig[:, 0:4, :], gg[:])
                    nc.vector.tensor_mul(cT[:, cc], sig[:, 4:8, :], cT[:, cp])
                    nc.vector.tensor_add(cT[:, cc], cT[:, cc], t1[:])
                    th = work.tile([128, 4, B], F32, tag="th")
                    nc.scalar.activation(th[:], cT[:, cc], AF.Tanh)
                    nc.vector.tensor_mul(hT[:, cur], sigo[:], th[:])
                    nc.sync.dma_start(d_oh[t], hT[:, cur])

                cur_xt, cur_xg = nxt_xt, nxt_xg

            nc.sync.dma_start(d_oc[:], cT[:, (T - 1) % 2])
    nc.compile()
    return nc


def _prep_core_inputs(word_seq, w_ih, w_hh, b_ih, b_hh, core):
    whh_t = w_hh.astype(BF)
    wih_t = w_ih.astype(BF)
    lhsT_hh = np.ascontiguousarray(
        whh_t.T.reshape(4, 128, 2048).transpose(1, 0, 2))
    lhsT_ih = np.ascontiguousarray(
        wih_t.T.reshape(4, 128, 2048).transpose(1, 0, 2))
    biasp = (b_ih + b_hh).astype(np.float32)
    bias = np.ascontiguousarray(biasp.reshape(16, 128).T)
    bs = word_seq[:, B * core:B * (core + 1), :]  # [T, B, I]
    a = bs.reshape(NCHUNK, TC, B, 4, 128).astype(BF)
    xT = np.ascontiguousarray(
        a.transpose(0, 4, 3, 1, 2).reshape(NCHUNK, 128, 4, TC * B))
    return {"lhsT_hh": lhsT_hh, "lhsT_ih": lhsT_ih, "bias": bias, "xT": xT}


def kernel(word_seq, w_ih, w_hh, b_ih, b_hh):
    global _compiled_nc
    from concourse.bass_utils import run_bass_kernel_spmd

    word_seq = np.asarray(word_seq, dtype=np.float32)
    w_ih = np.asarray(w_ih, dtype=np.float32)
    w_hh = np.asarray(w_hh, dtype=np.float32)
    b_ih = np.asarray(b_ih, dtype=np.float32)
    b_hh = np.asarray(b_hh, dtype=np.float32)
    assert word_seq.shape == (T, BTOT, I), word_seq.shape

    if _compiled_nc is None:
        _compiled_nc = _build()
    nc = _compiled_nc
    in_maps = [_prep_core_inputs(word_seq, w_ih, w_hh, b_ih, b_hh, c)
               for c in range(N_CORES)]
    res = run_bass_kernel_spmd(nc, in_maps, core_ids=list(range(N_CORES)))
    outs = res.results

    out = np.empty((T, BTOT, H), np.float32)
    c_fin = np.empty((BTOT, H), np.float32)
    for ci in range(N_CORES):
        oh = np.asarray(outs[ci]["out_h"]).astype(np.float32)  # [T,128,4,B]
        out[:, B * ci:B * (ci + 1), :] = oh.transpose(0, 3, 2, 1).reshape(T, B, H)
        oc = np.asarray(outs[ci]["out_c"])                     # [128,4,B]
        c_fin[B * ci:B * (ci + 1), :] = oc.transpose(2, 1, 0).reshape(B, H)
    h_fin = out[-1].copy()
    return out, (h_fin, c_fin)


# revision 9
# speedup vs baseline: 1.0014x; 1.0014x over previous
"""TRN2 Bass LSTM kernel — batch-data-parallel across 8 NeuronCores.

Problem: nn_CCellBaseLSTM (T=2048, B=64, I=H=512).
  out[t] = h_t of a standard LSTM; returns (out, (h_T, c_T)) like the reference.

Design (per core, B=8 batch lanes, everything "transposed" so gate rows /
hidden units sit on SBUF partitions):
- Input GEMM is chunked over time (TC=128): xg.T = w_ih' @ x.T streamed as
  bf16 matmuls with the weights stationary, bias folded into the psum->SBUF
  copy (DVE tensor_scalar add with a per-partition bias operand).
- Recurrence: gates.T [2048,8] per step as 64 (LDWEIGHTS+MATMUL) pairs with
  K=128, M=128, N=8, accumulating 4 K-tiles per gate row tile into PSUM.
  PSUM is split across two banks (i,f,g vs o) so the activation chain can
  start while the o-gate matmuls finish.
- Elementwise: ACT does sigmoid/tanh, DVE does the c/h updates in f32;
  h is written as bf16 (matmul operand dtype) and DMA'd out per step.
- Host does all layout transforms (weight transpose/reshape, x transpose to
  [I, T, B], output unpermute) — only device time counts.
"""
import os
import sys

for _p in ("/opt/trn_rl_repo", "/root/.axon_site/_ro/trn_rl_repo"):
    if os.path.isdir(_p) and _p not in sys.path:
        sys.path.insert(0, _p)

import numpy as np
import ml_dtypes

T, BTOT, I, H = 2048, 64, 512, 512
N_CORES = 8
B = BTOT // N_CORES          # 8 batch lanes per core
TC = 64                      # timesteps per chunk
NCHUNK = T // TC
BF = ml_dtypes.bfloat16

_compiled_nc = None


def _build():
    import concourse.mybir as mybir
    import concourse.tile as tile
    from concourse import bacc

    F32 = mybir.dt.float32
    BF16 = mybir.dt.bfloat16
    AF = mybir.ActivationFunctionType
    NCOL = TC * B

    nc = bacc.Bacc("TRN2", target_bir_lowering=False, debug=False,
                   num_devices=N_CORES)
    d_whh = nc.dram_tensor("lhsT_hh", [128, 4, 2048], BF16, kind="ExternalInput")
    d_wih = nc.dram_tensor("lhsT_ih", [128, 4, 2048], BF16, kind="ExternalInput")
    d_bias = nc.dram_tensor("bias", [128, 16], F32, kind="ExternalInput")
    d_xT = nc.dram_tensor("xT", [NCHUNK, 128, 4, NCOL], BF16, kind="ExternalInput")
    d_oh = nc.dram_tensor("out_h", [T, 128, 4, B], BF16, kind="ExternalOutput")
    d_oc = nc.dram_tensor("out_c", [128, 4, B], F32, kind="ExternalOutput")

    HDEPTH = 4  # h ring depth (lets PE run ahead of the act chain)
    with tile.TileContext(nc) as tc:
        with (
            tc.tile_pool(name="const", bufs=1) as const,
            tc.tile_pool(name="state", bufs=1) as state,
            tc.tile_pool(name="xin", bufs=2) as xpool,
            tc.tile_pool(name="xg", bufs=2) as xgpool,
            tc.tile_pool(name="psx", bufs=2, space="PSUM") as psx,
            tc.tile_pool(name="psg", bufs=2, space="PSUM") as psg,
            tc.tile_pool(name="work", bufs=4) as work,
        ):
            whh = const.tile([128, 4, 2048], BF16)
            wih = const.tile([128, 4, 2048], BF16)
            bias = const.tile([128, 16], F32)
            nc.sync.dma_start(whh[:], d_whh[:])
            nc.sync.dma_start(wih[:], d_wih[:])
            nc.sync.dma_start(bias[:], d_bias[:])

            cT = state.tile([128, 2, 4, B], F32)   # ping-pong c state
            hT = state.tile([128, HDEPTH, 4, B], BF16)
            nc.vector.memset(cT[:], 0.0)
            nc.vector.memset(hT[:], 0.0)

            NSPL = max(1, NCOL // 512)  # 512-col splits per M-tile
            NGRP = 16 * NSPL            # x-GEMM (j, s) groups per chunk
            xstate = {}

            def emit_xgemm_group(xt_t, xg_t, gidx):
                j, s = gidx // NSPL, (gidx % NSPL) * 512
                w = min(512, NCOL - s)
                pt = psx.tile([128, 512], F32, name="xq_pt", tag="xq_pt")
                for k in range(4):
                    nc.tensor.matmul(
                        pt[:, :w],
                        lhsT=wih[:, k, 128 * j:128 * (j + 1)],
                        rhs=xt_t[:, k, s:s + w],
                        start=(k == 0), stop=(k == 3),
                    )
                nc.vector.tensor_scalar_add(
                    xg_t[:, j, s:s + w], pt[:, :w], bias[:, j:j + 1])

            def emit_xgemm_quarter(xt_t, xg_t, qidx):
                """One K-slice of one (j, s) group: spreads a group's 4 MMs
                over 4 successive steps to avoid bursty PE injection."""
                gidx, k = qidx // 4, qidx % 4
                if gidx >= NGRP:
                    return
                j, s = gidx // NSPL, (gidx % NSPL) * 512
                w = min(512, NCOL - s)
                if k == 0:
                    xstate["pt"] = psx.tile([128, 512], F32, name="xq_pt",
                                            tag="xq_pt")
                pt = xstate["pt"]
                nc.tensor.matmul(
                    pt[:, :w],
                    lhsT=wih[:, k, 128 * j:128 * (j + 1)],
                    rhs=xt_t[:, k, s:s + w],
                    start=(k == 0), stop=(k == 3),
                )
                if k == 3:
                    nc.vector.tensor_scalar_add(
                        xg_t[:, j, s:s + w], pt[:, :w], bias[:, j:j + 1])

            # prologue: chunk 0 input GEMM up front
            cur_xt = xpool.tile([128, 4, NCOL], BF16, tag="xt")
            nc.sync.dma_start(cur_xt[:], d_xT[0])
            cur_xg = xgpool.tile([128, 16, NCOL], BF16, tag="xg")
            for g in range(NGRP):
                emit_xgemm_group(cur_xt, cur_xg, g)

            for n in range(NCHUNK):
                xg = cur_xg
                nxt_xt = nxt_xg = None
                if n + 1 < NCHUNK:
                    nxt_xt = xpool.tile([128, 4, NCOL], BF16, tag="xt")
                    nc.sync.dma_start(nxt_xt[:], d_xT[n + 1])
                    nxt_xg = xgpool.tile([128, 16, NCOL], BF16, tag="xg")

                for tt in range(TC):
                    t = n * TC + tt
                    cur = t % HDEPTH
                    prv = (t - 1) % HDEPTH
                    # gate order in M-tiles: i=0:4, f=4:8, g=8:12, o=12:16
                    def mm_block(dst, jlist):
                        for ji, j in enumerate(jlist):
                            for k in range(4):
                                nc.tensor.matmul(
                                    dst[:, ji, :],
                                    lhsT=whh[:, k, 128 * j:128 * (j + 1)],
                                    rhs=hT[:, prv, k, :],
                                    start=(k == 0), stop=(k == 3),
                                )
                    xgs = xg[:, :, tt * B:(tt + 1) * B]
                    # g-gates first in their own PSUM bank so tanh(g)
                    # overlaps the i/f matmuls; o last (only needed for h)
                    pg_g = psg.tile([128, 4, B], F32, tag="pg_g")
                    mm_block(pg_g, [8, 9, 10, 11])
                    gpre = work.tile([128, 4, B], F32, tag="gpre")
                    nc.vector.tensor_tensor(
                        gpre[:], pg_g[:], xgs[:, 8:12, :], mybir.AluOpType.add)
                    gg = work.tile([128, 4, B], F32, tag="gg")
                    nc.scalar.activation(gg[:], gpre[:], AF.Tanh)
                    pg_if = psg.tile([128, 8, B], F32, tag="pg_if")
                    mm_block(pg_if, [0, 1, 2, 3, 4, 5, 6, 7])
                    pre = work.tile([128, 8, B], F32, tag="pre")
                    nc.vector.tensor_tensor(
                        pre[:], pg_if[:], xgs[:, 0:8, :], mybir.AluOpType.add)
                    sig = work.tile([128, 8, B], F32, tag="sig")
                    nc.scalar.activation(sig[:], pre[:], AF.Sigmoid)
                    pg_o = psg.tile([128, 4, B], F32, tag="pg_o")
                    mm_block(pg_o, [12, 13, 14, 15])
                    # next chunk's input GEMM, spread 1 matmul per step and
                    # emitted after the h-dependent matmuls so it fills PE
                    # idle during the activation chain
                    if nxt_xg is not None:
                        emit_xgemm_quarter(nxt_xt, nxt_xg, tt)
                    preo = work.tile([128, 4, B], F32, tag="preo")
                    nc.vector.tensor_tensor(
                        preo[:], pg_o[:], xgs[:, 12:16, :], mybir.AluOpType.add)
                    sigo = work.tile([128, 4, B], F32, tag="sigo")
                    nc.scalar.activation(sigo[:], preo[:], AF.Sigmoid)
                    cc, cp = t % 2, 1 - (t % 2)   # c ping-pong slots
                    t1 = work.tile([128, 4, B], F32, tag="t1")
                    nc.vector.tensor_mul(t1[:], sig[:, 0:4, :], gg[:])
                    nc.vector.tensor_mul(cT[:, cc], sig[:, 4:8, :], cT[:, cp])
                    nc.vector.tensor_add(cT[:, cc], cT[:, cc], t1[:])
                    th = work.tile([128, 4, B], F32, tag="th")
                    nc.scalar.activation(th[:], cT[:, cc], AF.Tanh)
                    nc.vector.tensor_mul(hT[:, cur], sigo[:], th[:])
                    nc.sync.dma_start(d_oh[t], hT[:, cur])

                cur_xt, cur_xg = nxt_xt, nxt_xg

            nc.sync.dma_start(d_oc[:], cT[:, (T - 1) % 2])
    nc.compile()
    return nc


def _prep_core_inputs(word_seq, w_ih, w_hh, b_ih, b_hh, core):
    whh_t = w_hh.astype(BF)
    wih_t = w_ih.astype(BF)
    lhsT_hh = np.ascontiguousarray(
        whh_t.T.reshape(4, 128, 2048).transpose(1, 0, 2))
    lhsT_ih = np.ascontiguousarray(
        wih_t.T.reshape(4, 128, 2048).transpose(1, 0, 2))
    biasp = (b_ih + b_hh).astype(np.float32)
    bias = np.ascontiguousarray(biasp.reshape(16, 128).T)
    bs = word_seq[:, B * core:B * (core + 1), :]  # [T, B, I]
    a = bs.reshape(NCHUNK, TC, B, 4, 128).astype(BF)
    xT = np.ascontiguousarray(
        a.transpose(0, 4, 3, 1, 2).reshape(NCHUNK, 128, 4, TC * B))
    return {"lhsT_hh": lhsT_hh, "lhsT_ih": lhsT_ih, "bias": bias, "xT": xT}


def kernel(word_seq, w_ih, w_hh, b_ih, b_hh):
    global _compiled_nc
    from concourse.bass_utils import run_bass_kernel_spmd

    word_seq = np.asarray(word_seq, dtype=np.float32)
    w_ih = np.asarray(w_ih, dtype=np.float32)
    w_hh = np.asarray(w_hh, dtype=np.float32)
    b_ih = np.asarray(b_ih, dtype=np.float32)
    b_hh = np.asarray(b_hh, dtype=np.float32)
    assert word_seq.shape == (T, BTOT, I), word_seq.shape

    if _compiled_nc is None:
        _compiled_nc = _build()
    nc = _compiled_nc
    in_maps = [_prep_core_inputs(word_seq, w_ih, w_hh, b_ih, b_hh, c)
               for c in range(N_CORES)]
    res = run_bass_kernel_spmd(nc, in_maps, core_ids=list(range(N_CORES)))
    outs = res.results

    out = np.empty((T, BTOT, H), np.float32)
    c_fin = np.empty((BTOT, H), np.float32)
    for ci in range(N_CORES):
        oh = np.asarray(outs[ci]["out_h"]).astype(np.float32)  # [T,128,4,B]
        out[:, B * ci:B * (ci + 1), :] = oh.transpose(0, 3, 2, 1).reshape(T, B, H)
        oc = np.asarray(outs[ci]["out_c"])                     # [128,4,B]
        c_fin[B * ci:B * (ci + 1), :] = oc.transpose(2, 1, 0).reshape(B, H)
    h_fin = out[-1].copy()
    return out, (h_fin, c_fin)
